# revision 32
# baseline (speedup 1.0000x reference)
"""Attention-pooling kernel for Trainium2 (8 NeuronCores, data parallel).

Computes, for full inputs query [B, D], keys [B, L, D], W [1, D]:
    inter  = keys * query[:, None, :]
    scores = tanh(einsum('bld,od->blo', inter, W))
    p      = softmax(scores, axis=1)
    out    = sum(p * keys, axis=1)                      # [B, D]

Sharding: batch dim split evenly across 8 cores; W replicated.

Fast path: keys are transposed to [B, D, L] and cast to bf16 on the host, so
each [128, D*L] SBUF tile has contiguous l-runs. Scores come from 64
accumulating rank-1 diag matmuls (four 32x32 tile_position blocks, one PSUM
bank each, so the block streams run concurrently at full rhs rate); ACT does
tanh/exp (+ softmax denominator via accum_out); DVE does the exp-weight
multiply (outer-dim broadcast, 2x mode) and an alignment-preserving halving
add-tree over l. Keys stream via chunked SWDGE DMAs double-buffered 3 deep;
the next tile's query load and diag build are software-pipelined one tile
ahead so the PE always has weights ready.
A scaled-absmax spot check falls back to the f32 SAFE variant on failure.
"""

import sys

if "/opt/trn_rl_repo" not in sys.path:
    sys.path.insert(0, "/opt/trn_rl_repo")

import numpy as np

import concourse.bacc as bacc
import concourse.bass as bass
import concourse.mybir as mybir
import concourse.tile as tile
from concourse.bass_utils import run_bass_kernel_spmd

B, L, D = 16384, 200, 64
NCORES = 8
BC = B // NCORES  # batch rows per core
PT = 128          # partition tile (batch rows per SBUF tile)
NT = BC // PT     # tiles per core

# variant = (keys_bf16, d_tree, l_tree, pe2, rk1, hwcast, rk4, pbank, kT, v2,
#            dchunk, actoff, kbf, khw, gpoff, rsum, rtail, kb4)
# FAST: d-major host-transposed bf16 keys, 32x32 diag-block PE scoring with
# per-block PSUM banks, chunked SWDGE loads, aligned DVE add-tree.
FAST_VARIANT = (
    True, False, True, False, True, False, True, True, True, True, True,
    False, True, False, False, False, False, False, True,
)
SAFE_VARIANT = (False, False, False)
DEFAULT_VARIANT = SAFE_VARIANT

_cache = {}
_run_state = {"variant": FAST_VARIANT, "checked": False}


def _tree_reduce_outer(nc, pool, src_ap, n_outer, inner, dtype, out_ap, tag):
    """Sum over the OUTER axis of a [PT, n_outer, inner] view via halving
    tensor_tensor adds (inner dim stays contiguous, 2x-mode eligible for
    bf16). Final [PT, inner] f32 result lands in out_ap."""
    cur = src_ap
    n = n_outer
    lvl = 0
    while n > 1:
        h, odd = n // 2, n % 2
        if h + odd == 1:
            nc.vector.tensor_add(
                out_ap.unsqueeze(1), cur[:, 0:1, :], cur[:, 1:2, :]
            )
            return
        # ping-pong tags: level k+1 reads level k, so they must coexist
        t = pool.tile([PT, (h + odd) * inner], dtype, tag=f"{tag}{lvl % 2}")
        dst = t[:].rearrange("p (n i) -> p n i", n=h + odd)
        nc.vector.tensor_add(dst[:, 0:h, :], cur[:, 0:h, :], cur[:, h : 2 * h, :])
        if odd:
            nc.vector.tensor_copy(dst[:, h : h + 1, :], cur[:, 2 * h : n, :])
        cur = dst
        n = h + odd
        lvl += 1


# Halving schedule for n=200 chosen so every level's run stride and second
# operand offset stay 4-byte aligned (bf16), keeping DVE 2x mode: entries are
# (h, n_copy) -> n_next = h + n_copy.
_SCHED_200 = [(100, 0), (50, 0), (24, 2), (12, 2), (6, 2), (4, 0), (2, 0), (1, 0)]


def _tree_reduce_partial(nc, pool, src_ap, outer, sched, dtype, tag):
    """Run the first len(sched) halving levels and return the current
    [PT, outer, n] view for another engine to finish."""
    cur = src_ap
    for lvl, (h, cp) in enumerate(sched):
        t = pool.tile([PT, outer * (h + cp)], dtype, tag=f"{tag}{lvl % 2}")
        dst = t[:].rearrange("p (o i) -> p o i", o=outer)
        nc.vector.tensor_add(dst[:, :, 0:h], cur[:, :, 0:h], cur[:, :, h : 2 * h])
        if cp:
            nc.vector.tensor_copy(
                dst[:, :, h : h + cp], cur[:, :, 2 * h : 2 * h + cp]
            )
        cur = dst
    return cur


def _tree_reduce_inner_sched(nc, pool, src_ap, outer, sched, dtype, out_ap, tag):
    """Like _tree_reduce_inner_any but with an explicit (h, n_copy) level
    schedule keeping all adds 2x-eligible."""
    cur = src_ap
    lvl = 0
    for h, cp in sched:
        if h == 1:
            nc.vector.tensor_add(
                out_ap.unsqueeze(2), cur[:, :, 0:1], cur[:, :, 1:2]
            )
            return
        t = pool.tile([PT, outer * (h + cp)], dtype, tag=f"{tag}{lvl % 2}")
        dst = t[:].rearrange("p (o i) -> p o i", o=outer)
        nc.vector.tensor_add(dst[:, :, 0:h], cur[:, :, 0:h], cur[:, :, h : 2 * h])
        if cp:
            nc.vector.tensor_copy(
                dst[:, :, h : h + cp], cur[:, :, 2 * h : 2 * h + cp]
            )
        cur = dst
        lvl += 1


def _tree_reduce_inner_any(nc, pool, src_ap, outer, n_inner, dtype, out_ap, tag):
    """Sum over the INNER axis of a [PT, outer, n_inner] view via halving
    tensor_tensor adds on contiguous inner slices; odd levels park the
    leftover element with a copy. Final [PT, outer] f32 result in out_ap."""
    cur = src_ap
    n = n_inner
    lvl = 0
    while n > 1:
        h, odd = n // 2, n % 2
        if h + odd == 1:
            nc.vector.tensor_add(
                out_ap.unsqueeze(2), cur[:, :, 0:1], cur[:, :, 1:2]
            )
            return
        t = pool.tile([PT, outer * (h + odd)], dtype, tag=f"{tag}{lvl % 2}")
        dst = t[:].rearrange("p (o i) -> p o i", o=outer)
        nc.vector.tensor_add(dst[:, :, 0:h], cur[:, :, 0:h], cur[:, :, h : 2 * h])
        if odd:
            nc.vector.tensor_copy(dst[:, :, h : h + 1], cur[:, :, 2 * h : n])
        cur = dst
        n = h + odd
        lvl += 1


def _tree_reduce_inner(nc, pool, src_ap, outer, n_inner, dtype, out_ap, tag):
    """Sum over the INNER axis of a [PT, outer, n_inner] view via halving
    tensor_tensor adds on contiguous inner slices. n_inner must be a power
    of two. Final [PT, outer] f32 result lands in out_ap."""
    cur = src_ap
    n = n_inner
    lvl = 0
    while n > 1:
        h = n // 2
        if h == 1:
            nc.vector.tensor_add(
                out_ap.unsqueeze(2), cur[:, :, 0:1], cur[:, :, 1:2]
            )
            return
        t = pool.tile([PT, outer * h], dtype, tag=f"{tag}{lvl % 2}")
        dst = t[:].rearrange("p (o i) -> p o i", o=outer)
        nc.vector.tensor_add(dst, cur[:, :, 0:h], cur[:, :, h:n])
        cur = dst
        n = h
        lvl += 1


def _build_bass(variant):
    keys_bf16, d_tree, l_tree = variant[:3]
    pe2 = variant[3] if len(variant) > 3 else False
    rk1 = variant[4] if len(variant) > 4 else False
    hwcast = variant[5] if len(variant) > 5 else False  # f32 HWDGE load + DVE convert
    rk4 = variant[6] if len(variant) > 6 else False  # 32x32 block-diag tile_position
    pbank = variant[7] if len(variant) > 7 else False  # per-block PSUM banks
    kT = variant[8] if len(variant) > 8 else False  # host-transposed keys [BC, D, L]
    v2 = variant[9] if len(variant) > 9 else False  # aligned tree schedule
    dchunk = variant[10] if len(variant) > 10 else False  # chunked keys DMA
    actoff = variant[11] if len(variant) > 11 else False  # v2p/of on ACT
    kbf = variant[12] if len(variant) > 12 else False  # host-precast bf16 keys in HBM
    khw = variant[13] if len(variant) > 13 else False  # keys via sync HWDGE + prefetch
    gpoff = variant[14] if len(variant) > 14 else False  # gpsimd offloads
    rsum = variant[15] if len(variant) > 15 else False  # single 2x reduce_sum over l
    rtail = variant[16] if len(variant) > 16 else False  # tree to n=26, reduce the rest
    kb4 = variant[17] if len(variant) > 17 else False  # keys pool 4 buffers
    pipe = variant[18] if len(variant) > 18 else False  # hoist next tile's front
    pnorm = variant[19] if len(variant) > 19 else False  # scale exp by 1/S pre-mult
    assert not khw or kbf, "khw needs bf16 keys (no cast)"
    assert not pbank or rk4, "pbank requires rk4 blocks"
    assert not rk1 or keys_bf16, "rank-1 scoring requires bf16 keys"
    assert not kT or (rk1 and rk4 and pbank and not pe2), "kT path fixes the rest"
    assert not (v2 or dchunk or actoff) or kT, "v2 flags build on kT"
    f32 = mybir.dt.float32
    bf16 = mybir.dt.bfloat16
    kdt = bf16 if keys_bf16 else f32
    mdt = bf16 if keys_bf16 else f32  # multiply output dtype
    AF = mybir.ActivationFunctionType
    X = mybir.AxisListType.X

    nc = bacc.Bacc("TRN2", target_bir_lowering=False, debug=False, num_devices=NCORES)
    q_h = nc.declare_dram_parameter("query", [BC, D], f32, isOutput=False)
    k_h = nc.declare_dram_parameter(
        "keys",
        [BC, D, L] if kT else [BC, L, D],
        bf16 if kbf else f32,
        isOutput=False,
    )
    w_h = nc.declare_dram_parameter("W", [PT, D], f32, isOutput=False)
    if rk1:
        e_h = nc.declare_dram_parameter(
            "eye", [PT, 32 if rk4 else PT], bf16, isOutput=False
        )
    o_h = nc.declare_dram_parameter("out", [BC, D], f32, isOutput=True)

    with tile.TileContext(nc) as tc:
        with (
            tc.tile_pool(name="keys", bufs=2) as kp,
            tc.tile_pool(name="work", bufs=2) as wp,
            tc.tile_pool(name="tree", bufs=1) as tp,
            tc.tile_pool(name="small", bufs=2) as sp,
            tc.tile_pool(name="diag", bufs=3) as dgp,
            tc.tile_pool(name="psum", bufs=2, space="PSUM") as pp,
            tc.tile_pool(name="const", bufs=1) as cp,
        ):
            if rk1:
                ew = 32 if rk4 else PT
                eye0 = cp.tile([PT, ew], bf16)
                nc.sync.dma_start(eye0[:], e_h[:])
                eye_t = cp.tile([PT, ew], bf16)
                nc.vector.tensor_copy(eye_t[:], eye0[:])
            # W pre-broadcast to all 128 partitions on the host.
            wb0 = cp.tile([PT, D], f32)
            nc.sync.dma_start(wb0[:], w_h[:])
            # Route through a DVE copy so downstream DVE ops depend on it via
            # program order rather than an extra DMA semaphore wait.
            wb = cp.tile([PT, D], f32)
            nc.vector.tensor_copy(wb[:], wb0[:])

            _kt_next = [None]
            _front_next = [None]

            def _front(tt, sp, dgp, nc, q_h, wb, eye_t):
                """q load + diag-block build for tile tt (DVE front ops)."""
                f32 = mybir.dt.float32
                bf16 = mybir.dt.bfloat16
                trows = slice(tt * PT, (tt + 1) * PT)
                qt = sp.tile([PT, D], f32, tag="q", name="qt_f")
                nc.sync.dma_start(qt[:], q_h[trows, :])
                v2p = sp.tile([PT, 2 * D], bf16, tag="v2p", name="v2p_f")
                v2v = v2p[:].rearrange("p (d two) -> p d two", two=2)
                nc.vector.tensor_mul(v2v[:, :, 0], qt[:], wb[:])
                nc.vector.tensor_mul(v2v[:, :, 1], qt[:], wb[:])
                dga = dgp.tile([PT, D * 32], bf16, tag="dg", bufs=3, name="dga_f")
                nc.vector.tensor_mul(
                    dga[:].rearrange("p (d j2 two) -> p d j2 two", d=D, two=2),
                    eye_t[:]
                    .rearrange("p (j2 two) -> p j2 two", two=2)
                    .unsqueeze(1)
                    .broadcast_to([PT, D, 16, 2]),
                    v2v.unsqueeze(2).broadcast_to([PT, D, 16, 2]),
                )
                return dga

            for t in range(NT):
                rows = slice(t * PT, (t + 1) * PT)

                if kT:
                    # --- d-major keys path: keys pre-transposed to [BC, D, L]
                    # on the host, so the matmul rhs and the weighted-sum
                    # multiply both walk contiguous l-runs.
                    def _keys_dma(dst, trows, nchunks=4):
                        eng = nc.sync if khw else nc.gpsimd
                        if dchunk:
                            # chunked DMA: matmuls on early d-chunks start
                            # while later chunks stream in (cuts pipeline fill)
                            DC = D // nchunks
                            for c in range(nchunks):
                                eng.dma_start(
                                    dst[:, c * DC * L : (c + 1) * DC * L],
                                    k_h[trows, c * DC : (c + 1) * DC].rearrange(
                                        "b d l -> b (d l)"
                                    ),
                                )
                        else:
                            eng.dma_start(
                                dst[:], k_h[trows].rearrange("b d l -> b (d l)")
                            )

                    kbufs = 4 if kb4 else 3
                    if khw or gpoff:
                        # software prefetch: issue tile t+1's keys DMA at the
                        # top of tile t so queue waits never block the load
                        if t == 0:
                            kt = kp.tile([PT, L * D], kdt, tag="keys", bufs=kbufs)
                            _keys_dma(kt, rows, nchunks=16)
                        else:
                            kt = _kt_next[0]
                        if t + 1 < NT:
                            ktn = kp.tile(
                                [PT, L * D], kdt, tag="keys", bufs=kbufs, name="ktn"
                            )
                            _keys_dma(ktn, slice((t + 1) * PT, (t + 2) * PT))
                            _kt_next[0] = ktn
                    else:
                        kt = kp.tile([PT, L * D], kdt, tag="keys", bufs=kbufs)
                        _keys_dma(kt, rows)
                    k3t = kt[:].rearrange("p (d l) -> p d l", d=D)

                    if pipe:
                        if t == 0:
                            _front_next[0] = _front(0, sp, dgp, nc, q_h, wb, eye_t)
                        dga = _front_next[0]
                        dg3 = dga[:].rearrange("p (d j) -> p d j", d=D)
                    else:
                        qt = sp.tile([PT, D], f32, tag="q")
                        nc.sync.dma_start(qt[:], q_h[rows, :])

                        # v = q * W, duplicated into bf16 pairs for dga build
                        vt = sp.tile([PT, D], f32, tag="v")
                        nc.vector.tensor_mul(vt[:], qt[:], wb[:])
                    if pipe:
                        pass
                    elif gpoff:
                        dga = dgp.tile([PT, D * 32], bf16, tag="dg", bufs=3)
                        # gpsimd has no packed-mode alignment constraints:
                        # build the diag blocks straight from f32 v
                        nc.gpsimd.tensor_mul(
                            dga[:].rearrange("p (d j) -> p d j", d=D),
                            eye_t[:].unsqueeze(1).broadcast_to([PT, D, 32]),
                            vt[:].unsqueeze(2).broadcast_to([PT, D, 32]),
                        )
                    else:
                        dga = dgp.tile([PT, D * 32], bf16, tag="dg", bufs=3)
                        v2p = sp.tile([PT, 2 * D], bf16, tag="v2p")
                        v2v = v2p[:].rearrange("p (d two) -> p d two", two=2)
                        if actoff:
                            nc.scalar.activation(v2v[:, :, 0], vt[:], AF.Copy)
                            nc.scalar.activation(v2v[:, :, 1], vt[:], AF.Copy)
                        else:
                            nc.vector.tensor_copy(v2v[:, :, 0], vt[:])
                            nc.vector.tensor_copy(v2v[:, :, 1], vt[:])
                        nc.vector.tensor_mul(
                            dga[:].rearrange("p (d j2 two) -> p d j2 two", d=D, two=2),
                            eye_t[:]
                            .rearrange("p (j2 two) -> p j2 two", two=2)
                            .unsqueeze(1)
                            .broadcast_to([PT, D, 16, 2]),
                            v2v.unsqueeze(2).broadcast_to([PT, D, 16, 2]),
                        )
                    if not pipe:
                        dg3 = dga[:].rearrange("p (d j) -> p d j", d=D)

                    # scores: accumulating 32x32 diag-block matmuls, one PSUM
                    # bank per block so the four streams run concurrently
                    pscs = [
                        pp.tile([PT, L], f32, tag=f"sc{i}", name=f"psc{i}")
                        for i in range(4)
                    ]
                    for d in range(D):
                        for i in range(4):
                            s = slice(32 * i, 32 * i + 32)
                            nc.tensor.matmul(
                                pscs[i][s, :],
                                dg3[s, d, :],
                                k3t[s, d, :],
                                start=(d == 0),
                                stop=(d == D - 1),
                                tile_position=(32 * i, 32 * i),
                            )

                    th = sp.tile([PT, L], f32, tag="th")
                    for i in range(4):
                        s = slice(32 * i, 32 * i + 32)
                        nc.scalar.activation(th[s, :], pscs[i][s, :], AF.Tanh)
                    S = sp.tile([PT, 1], f32, tag="S")
                    pe = sp.tile([PT, L], bf16, tag="pe")
                    nc.scalar.activation(pe[:], th[:], AF.Exp, accum_out=S[:])
                    if pipe and t + 1 < NT:
                        _front_next[0] = _front(t + 1, sp, dgp, nc, q_h, wb, eye_t)
                    sinv = sp.tile([PT, 1], f32, tag="sinv")
                    nc.vector.reciprocal(sinv[:], S[:])
                    if pnorm:
                        # normalize the 200 weights once (bf16 4x) instead of
                        # the 64 outputs at the tail
                        pen = sp.tile([PT, L], bf16, tag="pen")
                        nc.vector.tensor_scalar_mul(pen[:], pe[:], sinv[:])
                        pe = pen

                    # wk[p, d, l] = keys * exp(scores); broadcast along outer d
                    wk = wp.tile([PT, L * D], bf16, tag="work")
                    w3t = wk[:].rearrange("p (d l) -> p d l", d=D)
                    if gpoff:
                        DS = D - 8  # gpsimd takes the top 8 d-values
                        nc.vector.tensor_mul(
                            w3t[:, 0:DS, :],
                            k3t[:, 0:DS, :],
                            pe[:].unsqueeze(1).broadcast_to([PT, DS, L]),
                        )
                        nc.gpsimd.tensor_mul(
                            w3t[:, DS:D, :],
                            k3t[:, DS:D, :],
                            pe[:].unsqueeze(1).broadcast_to([PT, D - DS, L]),
                        )
                    else:
                        nc.vector.tensor_mul(
                            w3t,
                            k3t,
                            pe[:].unsqueeze(1).broadcast_to([PT, D, L]),
                        )

                    # out_unnorm[p, d] = sum_l wk, then normalize
                    of = sp.tile([PT, D], f32, tag="of")
                    if rsum:
                        # contiguous bf16 l-runs: one 2x-mode reduce replaces
                        # the whole add-tree (fp32 internal accumulation)
                        ou = sp.tile([PT, D], bf16, tag="ou")
                        with nc.allow_low_precision(
                            reason="bf16 pooled output within 2e-2 tolerance"
                        ):
                            nc.vector.reduce_sum(ou[:], w3t, axis=X)
                    elif rtail:
                        # halve on the tree while runs are big; one reduce_sum
                        # mops up the overhead-dominated small levels
                        ou = sp.tile([PT, D], f32, tag="ou")
                        cur = _tree_reduce_partial(
                            nc, tp, w3t, D, _SCHED_200[:3], bf16, "ltree"
                        )
                        nc.vector.reduce_sum(ou[:], cur, axis=X)
                    elif v2:
                        ou = sp.tile([PT, D], f32, tag="ou")
                        _tree_reduce_inner_sched(
                            nc, tp, w3t, D, _SCHED_200, bf16, ou[:], "ltree"
                        )
                    else:
                        ou = sp.tile([PT, D], f32, tag="ou")
                        _tree_reduce_inner_any(
                            nc, tp, w3t, D, L, bf16, ou[:], "ltree"
                        )
                    if pnorm:
                        nc.sync.dma_start(o_h[rows, :], ou[:])
                    else:
                        nc.vector.tensor_scalar_mul(of[:], ou[:], sinv[:])
                        nc.sync.dma_start(o_h[rows, :], of[:])
                    continue

                kt = kp.tile(
                    [PT, L * D], kdt, tag="keys",
                    bufs=1 if hwcast else (3 if keys_bf16 else 2),
                )
                if keys_bf16 and hwcast:
                    ktf = kp.tile([PT, L * D], f32, tag="keysf")
                    nc.sync.dma_start(
                        ktf[:], k_h[rows].rearrange("b l d -> b (l d)")
                    )
                    nc.vector.tensor_copy(kt[:], ktf[:])
                elif keys_bf16:
                    # SWDGE cast-DMA: f32 HBM -> bf16 SBUF
                    nc.gpsimd.dma_start(
                        kt[:], k_h[rows].rearrange("b l d -> b (l d)")
                    )
                else:
                    nc.sync.dma_start(
                        kt[:], k_h[rows].rearrange("b l d -> b (l d)")
                    )
                qt = sp.tile([PT, D], f32, tag="q")
                nc.sync.dma_start(qt[:], q_h[rows, :])

                k3 = kt[:].rearrange("p (l d) -> p l d", l=L)

                if rk1:
                    # v = q * W kept f32, then duplicated into adjacent bf16
                    # pairs (v2p[2d], v2p[2d+1]) = v[d] for the paired
                    # broadcast below.
                    vt = sp.tile([PT, D], f32, tag="v")
                    nc.vector.tensor_mul(vt[:], qt[:], wb[:])
                    v2p = sp.tile([PT, 2 * D], bf16, tag="v2p")
                    v2v = v2p[:].rearrange("p (d two) -> p d two", two=2)
                    nc.vector.tensor_copy(v2v[:, :, 0], vt[:])
                    nc.vector.tensor_copy(v2v[:, :, 1], vt[:])
                    # Build all 64 diag(v[:, d]) blocks in one 2x-mode TT:
                    # dg_all[p, d, j] = eye[p, j] * v[p, d]
                    ew = 32 if rk4 else PT
                    dga = dgp.tile([PT, D * ew], bf16, tag="dg", bufs=1 if hwcast else 3)
                    nc.vector.tensor_mul(
                        dga[:].rearrange(
                            "p (d j2 two) -> p d j2 two", d=D, two=2
                        ),
                        eye_t[:]
                        .rearrange("p (j2 two) -> p j2 two", two=2)
                        .unsqueeze(1)
                        .broadcast_to([PT, D, ew // 2, 2]),
                        v2v.unsqueeze(2).broadcast_to([PT, D, ew // 2, 2]),
                    )
                    # scores[b, l] = sum_d v[b, d] * keys[b, l, d] as
                    # accumulating rank-1 diag matmuls on the TensorEngine:
                    # lhsT = diag(v[:, d]), rhs = keys[:, :, d]
                    dg3 = dga[:].rearrange("p (d j) -> p d j", d=D)
                    if pbank:
                        # one PSUM bank per 32-row block so the four
                        # tile_position matmuls can stream concurrently
                        pscs = [
                            pp.tile([PT, L], f32, tag=f"sc{i}", name=f"psc{i}")
                            for i in range(4)
                        ]
                        for d in range(D):
                            for i in range(4):
                                s = slice(32 * i, 32 * i + 32)
                                nc.tensor.matmul(
                                    pscs[i][s, :],
                                    dg3[s, d, :],
                                    k3[s, :, d],
                                    start=(d == 0),
                                    stop=(d == D - 1),
                                    tile_position=(32 * i, 32 * i),
                                )
                        scores = pscs
                    else:
                        psc = pp.tile([PT, L], f32, tag="sc")
                        for d in range(D):
                            if rk4:
                                # four concurrent 32x32 diag-block matmuls
                                for i in range(4):
                                    s = slice(32 * i, 32 * i + 32)
                                    nc.tensor.matmul(
                                        psc[s, :],
                                        dg3[s, d, :],
                                        k3[s, :, d],
                                        start=(d == 0),
                                        stop=(d == D - 1),
                                        tile_position=(32 * i, 32 * i),
                                    )
                            else:
                                nc.tensor.matmul(
                                    psc[:],
                                    dg3[:, d, :],
                                    k3[:, :, d],
                                    start=(d == 0),
                                    stop=(d == D - 1),
                                )
                        scores = psc
                else:
                    # v = q * W  (per-partition [128, 64])
                    vt = sp.tile([PT, D], mdt, tag="v")
                    nc.vector.tensor_mul(vt[:], qt[:], wb[:])

                    # inter = keys * v (v broadcast along l)
                    inter = wp.tile([PT, L * D], mdt, tag="work")
                    i3 = inter[:].rearrange("p (l d) -> p l d", l=L)
                    nc.vector.tensor_mul(
                        i3, k3, vt[:].unsqueeze(1).broadcast_to([PT, L, D])
                    )

                    # scores[b, l] = sum_d inter
                    scores = sp.tile([PT, L], f32, tag="sc")
                    if d_tree:
                        _tree_reduce_inner(nc, tp, i3, L, D, mdt, scores[:], "dtree")
                    else:
                        nc.vector.reduce_sum(scores[:], i3, axis=X)

                # tanh then exp (same ACT table set); accumulate softmax denom
                th = sp.tile([PT, L], f32, tag="th")
                if isinstance(scores, list):
                    for i in range(4):
                        s = slice(32 * i, 32 * i + 32)
                        nc.scalar.activation(th[s, :], scores[i][s, :], AF.Tanh)
                else:
                    nc.scalar.activation(th[:], scores[:], AF.Tanh)
                S = sp.tile([PT, 1], f32, tag="S")
                wk = wp.tile([PT, L * D], mdt, tag="work")
                w3 = wk[:].rearrange("p (l d) -> p l d", l=L)
                if pe2:
                    # exp weights duplicated into adjacent pairs so the
                    # broadcast-along-d AP has innermost step 1 (4B-aligned
                    # bf16 pair) -> DVE 2x_1P packed mode for the multiply.
                    ped = sp.tile([PT, 2 * L], mdt, tag="pe")
                    p3 = ped[:].rearrange("p (l two) -> p l two", two=2)
                    nc.scalar.activation(p3[:, :, 0], th[:], AF.Exp, accum_out=S[:])
                    nc.scalar.activation(p3[:, :, 1], th[:], AF.Exp)
                    sinv = sp.tile([PT, 1], f32, tag="sinv")
                    nc.vector.reciprocal(sinv[:], S[:])
                    nc.vector.tensor_mul(
                        wk[:].rearrange("p (l d2 two) -> p l d2 two", l=L, two=2),
                        kt[:].rearrange("p (l d2 two) -> p l d2 two", l=L, two=2),
                        p3.unsqueeze(2).broadcast_to([PT, L, D // 2, 2]),
                    )
                else:
                    pe = sp.tile([PT, L], mdt, tag="pe")
                    nc.scalar.activation(pe[:], th[:], AF.Exp, accum_out=S[:])
                    sinv = sp.tile([PT, 1], f32, tag="sinv")
                    nc.vector.reciprocal(sinv[:], S[:])
                    # wk = keys * exp(scores) (broadcast along d)
                    nc.vector.tensor_mul(
                        w3, k3, pe[:].unsqueeze(2).broadcast_to([PT, L, D])
                    )

                # out_unnorm[b, d] = sum_l wk
                ou = sp.tile([PT, D], f32, tag="ou")
                if l_tree:
                    _tree_reduce_outer(nc, tp, w3, L, D, mdt, ou[:], "ltree")
                else:
                    nc.vector.reduce_sum(
                        ou[:],
                        wk[:].rearrange("p (l d) -> p d l", l=L),
                        axis=X,
                    )
                # normalize by softmax denominator
                of = sp.tile([PT, D], f32, tag="of")
                nc.vector.tensor_scalar_mul(of[:], ou[:], sinv[:])
                nc.sync.dma_start(o_h[rows, :], of[:])

    nc.compile()
    return nc


def _get_nc(variant=DEFAULT_VARIANT):
    key = tuple(variant)
    if key not in _cache:
        _cache[key] = _build_bass(key)
    return _cache[key]


def run_sharded(query, keys, W, trace=False, variant=DEFAULT_VARIANT):
    """Run the SPMD kernel; returns (out [B, D], BassKernelResults)."""
    query = np.ascontiguousarray(query, dtype=np.float32)
    keys = np.ascontiguousarray(keys, dtype=np.float32)
    W = np.ascontiguousarray(W, dtype=np.float32)
    nc = _get_nc(variant)
    if len(variant) > 8 and variant[8]:
        # kT path: keys laid out [B, D, L] in HBM
        keys = np.ascontiguousarray(keys.transpose(0, 2, 1))
        if len(variant) > 12 and variant[12]:
            import ml_dtypes

            keys = keys.astype(ml_dtypes.bfloat16)
    w_b = np.ascontiguousarray(np.broadcast_to(W.reshape(1, D), (PT, D)))
    extra = {}
    if len(variant) > 4 and variant[4]:
        import ml_dtypes

        if len(variant) > 6 and variant[6]:
            e = np.zeros((PT, 32), dtype=ml_dtypes.bfloat16)
            e[np.arange(PT), np.arange(PT) % 32] = 1
            extra["eye"] = e
        else:
            extra["eye"] = np.eye(PT, dtype=ml_dtypes.bfloat16)
    in_maps = [
        {
            "query": query[i * BC : (i + 1) * BC],
            "keys": keys[i * BC : (i + 1) * BC],
            "W": w_b,
            **extra,
        }
        for i in range(NCORES)
    ]
    res = run_bass_kernel_spmd(nc, in_maps, core_ids=list(range(NCORES)), trace=trace)
    out = np.concatenate([res.results[i]["out"] for i in range(NCORES)], axis=0)
    return out, res


def _spot_check(out, query, keys, W, n=512):
    """Scaled absmax error of a row subset vs a float64 numpy oracle."""
    idx = np.random.default_rng(0).choice(B, n, replace=False)
    q = query[idx].astype(np.float64)
    k = keys[idx].astype(np.float64)
    w = W.reshape(-1).astype(np.float64)
    sc = np.tanh(((k * q[:, None, :]) * w).sum(-1))
    p = np.exp(sc)
    p /= p.sum(1, keepdims=True)
    ref = (p[:, :, None] * k).sum(1)
    return np.abs(out[idx] - ref).max() / max(np.abs(ref).max(), 1e-6)


def kernel(query, keys, W):
    var = _run_state["variant"]
    try:
        out, _ = run_sharded(query, keys, W, trace=False, variant=var)
        if var != SAFE_VARIANT and not _run_state["checked"]:
            _run_state["checked"] = True
            if _spot_check(out, query, keys, W) > 2e-2:
                raise RuntimeError("fast-variant accuracy check failed")
    except Exception:
        if var == SAFE_VARIANT:
            raise
        _run_state["variant"] = SAFE_VARIANT
        out, _ = run_sharded(query, keys, W, trace=False, variant=SAFE_VARIANT)
    return out



# revision 33
# speedup vs baseline: 1.0017x; 1.0017x over previous
"""Attention-pooling kernel for Trainium2 (8 NeuronCores, data parallel).

Computes, for full inputs query [B, D], keys [B, L, D], W [1, D]:
    inter  = keys * query[:, None, :]
    scores = tanh(einsum('bld,od->blo', inter, W))
    p      = softmax(scores, axis=1)
    out    = sum(p * keys, axis=1)                      # [B, D]

Sharding: batch dim split evenly across 8 cores; W replicated.

Fast path: keys are transposed to [B, D, L] and cast to bf16 on the host, so
each [128, D*L] SBUF tile has contiguous l-runs. Scores come from 64
accumulating rank-1 diag matmuls (four 32x32 tile_position blocks, one PSUM
bank each, so the block streams run concurrently at full rhs rate); ACT does
tanh/exp (+ softmax denominator via accum_out); DVE does the exp-weight
multiply (outer-dim broadcast, 2x mode) and an alignment-preserving halving
add-tree over l. Keys stream via chunked SWDGE DMAs double-buffered 3 deep;
the next tile's query load and diag build are software-pipelined one tile
ahead so the PE always has weights ready.
A scaled-absmax spot check falls back to the f32 SAFE variant on failure.
"""

import sys

if "/opt/trn_rl_repo" not in sys.path:
    sys.path.insert(0, "/opt/trn_rl_repo")

import numpy as np

import concourse.bacc as bacc
import concourse.bass as bass
import concourse.mybir as mybir
import concourse.tile as tile
from concourse.bass_utils import run_bass_kernel_spmd

B, L, D = 16384, 200, 64
NCORES = 8
BC = B // NCORES  # batch rows per core
PT = 128          # partition tile (batch rows per SBUF tile)
NT = BC // PT     # tiles per core

# variant = (keys_bf16, d_tree, l_tree, pe2, rk1, hwcast, rk4, pbank, kT, v2,
#            dchunk, actoff, kbf, khw, gpoff, rsum, rtail, kb4)
# FAST: d-major host-transposed bf16 keys, 32x32 diag-block PE scoring with
# per-block PSUM banks, chunked SWDGE loads, aligned DVE add-tree.
FAST_VARIANT = (
    True, False, True, False, True, False, True, True, True, True, True,
    False, True, False, False, False, False, False, True,
)
SAFE_VARIANT = (False, False, False)
DEFAULT_VARIANT = SAFE_VARIANT

_cache = {}
_run_state = {"variant": FAST_VARIANT, "checked": False}


def _tree_reduce_outer(nc, pool, src_ap, n_outer, inner, dtype, out_ap, tag):
    """Sum over the OUTER axis of a [PT, n_outer, inner] view via halving
    tensor_tensor adds (inner dim stays contiguous, 2x-mode eligible for
    bf16). Final [PT, inner] f32 result lands in out_ap."""
    cur = src_ap
    n = n_outer
    lvl = 0
    while n > 1:
        h, odd = n // 2, n % 2
        if h + odd == 1:
            nc.vector.tensor_add(
                out_ap.unsqueeze(1), cur[:, 0:1, :], cur[:, 1:2, :]
            )
            return
        # ping-pong tags: level k+1 reads level k, so they must coexist
        t = pool.tile([PT, (h + odd) * inner], dtype, tag=f"{tag}{lvl % 2}")
        dst = t[:].rearrange("p (n i) -> p n i", n=h + odd)
        nc.vector.tensor_add(dst[:, 0:h, :], cur[:, 0:h, :], cur[:, h : 2 * h, :])
        if odd:
            nc.vector.tensor_copy(dst[:, h : h + 1, :], cur[:, 2 * h : n, :])
        cur = dst
        n = h + odd
        lvl += 1


# Halving schedule for n=200 chosen so every level's run stride and second
# operand offset stay 4-byte aligned (bf16), keeping DVE 2x mode: entries are
# (h, n_copy) -> n_next = h + n_copy.
_SCHED_200 = [(100, 0), (50, 0), (24, 2), (12, 2), (6, 2), (4, 0), (2, 0), (1, 0)]


def _tree_reduce_partial(nc, pool, src_ap, outer, sched, dtype, tag):
    """Run the first len(sched) halving levels and return the current
    [PT, outer, n] view for another engine to finish."""
    cur = src_ap
    for lvl, (h, cp) in enumerate(sched):
        t = pool.tile([PT, outer * (h + cp)], dtype, tag=f"{tag}{lvl % 2}")
        dst = t[:].rearrange("p (o i) -> p o i", o=outer)
        nc.vector.tensor_add(dst[:, :, 0:h], cur[:, :, 0:h], cur[:, :, h : 2 * h])
        if cp:
            nc.vector.tensor_copy(
                dst[:, :, h : h + cp], cur[:, :, 2 * h : 2 * h + cp]
            )
        cur = dst
    return cur


def _tree_reduce_inner_sched(nc, pool, src_ap, outer, sched, dtype, out_ap, tag):
    """Like _tree_reduce_inner_any but with an explicit (h, n_copy) level
    schedule keeping all adds 2x-eligible."""
    cur = src_ap
    lvl = 0
    for h, cp in sched:
        if h == 1:
            nc.vector.tensor_add(
                out_ap.unsqueeze(2), cur[:, :, 0:1], cur[:, :, 1:2]
            )
            return
        t = pool.tile([PT, outer * (h + cp)], dtype, tag=f"{tag}{lvl % 2}")
        dst = t[:].rearrange("p (o i) -> p o i", o=outer)
        nc.vector.tensor_add(dst[:, :, 0:h], cur[:, :, 0:h], cur[:, :, h : 2 * h])
        if cp:
            nc.vector.tensor_copy(
                dst[:, :, h : h + cp], cur[:, :, 2 * h : 2 * h + cp]
            )
        cur = dst
        lvl += 1


def _tree_reduce_inner_any(nc, pool, src_ap, outer, n_inner, dtype, out_ap, tag):
    """Sum over the INNER axis of a [PT, outer, n_inner] view via halving
    tensor_tensor adds on contiguous inner slices; odd levels park the
    leftover element with a copy. Final [PT, outer] f32 result in out_ap."""
    cur = src_ap
    n = n_inner
    lvl = 0
    while n > 1:
        h, odd = n // 2, n % 2
        if h + odd == 1:
            nc.vector.tensor_add(
                out_ap.unsqueeze(2), cur[:, :, 0:1], cur[:, :, 1:2]
            )
            return
        t = pool.tile([PT, outer * (h + odd)], dtype, tag=f"{tag}{lvl % 2}")
        dst = t[:].rearrange("p (o i) -> p o i", o=outer)
        nc.vector.tensor_add(dst[:, :, 0:h], cur[:, :, 0:h], cur[:, :, h : 2 * h])
        if odd:
            nc.vector.tensor_copy(dst[:, :, h : h + 1], cur[:, :, 2 * h : n])
        cur = dst
        n = h + odd
        lvl += 1


def _tree_reduce_inner(nc, pool, src_ap, outer, n_inner, dtype, out_ap, tag):
    """Sum over the INNER axis of a [PT, outer, n_inner] view via halving
    tensor_tensor adds on contiguous inner slices. n_inner must be a power
    of two. Final [PT, outer] f32 result lands in out_ap."""
    cur = src_ap
    n = n_inner
    lvl = 0
    while n > 1:
        h = n // 2
        if h == 1:
            nc.vector.tensor_add(
                out_ap.unsqueeze(2), cur[:, :, 0:1], cur[:, :, 1:2]
            )
            return
        t = pool.tile([PT, outer * h], dtype, tag=f"{tag}{lvl % 2}")
        dst = t[:].rearrange("p (o i) -> p o i", o=outer)
        nc.vector.tensor_add(dst, cur[:, :, 0:h], cur[:, :, h:n])
        cur = dst
        n = h
        lvl += 1


def _build_bass(variant):
    keys_bf16, d_tree, l_tree = variant[:3]
    pe2 = variant[3] if len(variant) > 3 else False
    rk1 = variant[4] if len(variant) > 4 else False
    hwcast = variant[5] if len(variant) > 5 else False  # f32 HWDGE load + DVE convert
    rk4 = variant[6] if len(variant) > 6 else False  # 32x32 block-diag tile_position
    pbank = variant[7] if len(variant) > 7 else False  # per-block PSUM banks
    kT = variant[8] if len(variant) > 8 else False  # host-transposed keys [BC, D, L]
    v2 = variant[9] if len(variant) > 9 else False  # aligned tree schedule
    dchunk = variant[10] if len(variant) > 10 else False  # chunked keys DMA
    actoff = variant[11] if len(variant) > 11 else False  # v2p/of on ACT
    kbf = variant[12] if len(variant) > 12 else False  # host-precast bf16 keys in HBM
    khw = variant[13] if len(variant) > 13 else False  # keys via sync HWDGE + prefetch
    gpoff = variant[14] if len(variant) > 14 else False  # gpsimd offloads
    rsum = variant[15] if len(variant) > 15 else False  # single 2x reduce_sum over l
    rtail = variant[16] if len(variant) > 16 else False  # tree to n=26, reduce the rest
    kb4 = variant[17] if len(variant) > 17 else False  # keys pool 4 buffers
    pipe = variant[18] if len(variant) > 18 else False  # hoist next tile's front
    pnorm = variant[19] if len(variant) > 19 else False  # scale exp by 1/S pre-mult
    pipe2 = variant[20] if len(variant) > 20 else False  # front 2 tiles ahead
    assert not khw or kbf, "khw needs bf16 keys (no cast)"
    assert not pbank or rk4, "pbank requires rk4 blocks"
    assert not rk1 or keys_bf16, "rank-1 scoring requires bf16 keys"
    assert not kT or (rk1 and rk4 and pbank and not pe2), "kT path fixes the rest"
    assert not (v2 or dchunk or actoff) or kT, "v2 flags build on kT"
    f32 = mybir.dt.float32
    bf16 = mybir.dt.bfloat16
    kdt = bf16 if keys_bf16 else f32
    mdt = bf16 if keys_bf16 else f32  # multiply output dtype
    AF = mybir.ActivationFunctionType
    X = mybir.AxisListType.X

    nc = bacc.Bacc("TRN2", target_bir_lowering=False, debug=False, num_devices=NCORES)
    q_h = nc.declare_dram_parameter("query", [BC, D], f32, isOutput=False)
    k_h = nc.declare_dram_parameter(
        "keys",
        [BC, D, L] if kT else [BC, L, D],
        bf16 if kbf else f32,
        isOutput=False,
    )
    w_h = nc.declare_dram_parameter("W", [PT, D], f32, isOutput=False)
    if rk1:
        e_h = nc.declare_dram_parameter(
            "eye", [PT, 32 if rk4 else PT], bf16, isOutput=False
        )
    o_h = nc.declare_dram_parameter("out", [BC, D], f32, isOutput=True)

    with tile.TileContext(nc) as tc:
        with (
            tc.tile_pool(name="keys", bufs=2) as kp,
            tc.tile_pool(name="work", bufs=2) as wp,
            tc.tile_pool(name="tree", bufs=1) as tp,
            tc.tile_pool(name="small", bufs=2) as sp,
            tc.tile_pool(name="diag", bufs=3) as dgp,
            tc.tile_pool(name="psum", bufs=2, space="PSUM") as pp,
            tc.tile_pool(name="const", bufs=1) as cp,
        ):
            if rk1:
                ew = 32 if rk4 else PT
                eye0 = cp.tile([PT, ew], bf16)
                nc.sync.dma_start(eye0[:], e_h[:])
                eye_t = cp.tile([PT, ew], bf16)
                nc.vector.tensor_copy(eye_t[:], eye0[:])
            # W pre-broadcast to all 128 partitions on the host.
            wb0 = cp.tile([PT, D], f32)
            nc.sync.dma_start(wb0[:], w_h[:])
            # Route through a DVE copy so downstream DVE ops depend on it via
            # program order rather than an extra DMA semaphore wait.
            wb = cp.tile([PT, D], f32)
            nc.vector.tensor_copy(wb[:], wb0[:])

            _kt_next = [None]
            _front_next = [None, None]

            def _front(tt, sp, dgp, nc, q_h, wb, eye_t):
                """q load + diag-block build for tile tt (DVE front ops)."""
                f32 = mybir.dt.float32
                bf16 = mybir.dt.bfloat16
                trows = slice(tt * PT, (tt + 1) * PT)
                qt = sp.tile([PT, D], f32, tag="q", name="qt_f", bufs=3)
                nc.sync.dma_start(qt[:], q_h[trows, :])
                v2p = sp.tile([PT, 2 * D], bf16, tag="v2p", name="v2p_f", bufs=3)
                v2v = v2p[:].rearrange("p (d two) -> p d two", two=2)
                nc.vector.tensor_mul(v2v[:, :, 0], qt[:], wb[:])
                nc.vector.tensor_mul(v2v[:, :, 1], qt[:], wb[:])
                dga = dgp.tile([PT, D * 32], bf16, tag="dg", bufs=3, name="dga_f")
                nc.vector.tensor_mul(
                    dga[:].rearrange("p (d j2 two) -> p d j2 two", d=D, two=2),
                    eye_t[:]
                    .rearrange("p (j2 two) -> p j2 two", two=2)
                    .unsqueeze(1)
                    .broadcast_to([PT, D, 16, 2]),
                    v2v.unsqueeze(2).broadcast_to([PT, D, 16, 2]),
                )
                return dga

            for t in range(NT):
                rows = slice(t * PT, (t + 1) * PT)

                if kT:
                    # --- d-major keys path: keys pre-transposed to [BC, D, L]
                    # on the host, so the matmul rhs and the weighted-sum
                    # multiply both walk contiguous l-runs.
                    def _keys_dma(dst, trows, nchunks=4):
                        eng = nc.sync if khw else nc.gpsimd
                        if dchunk:
                            # chunked DMA: matmuls on early d-chunks start
                            # while later chunks stream in (cuts pipeline fill)
                            DC = D // nchunks
                            for c in range(nchunks):
                                eng.dma_start(
                                    dst[:, c * DC * L : (c + 1) * DC * L],
                                    k_h[trows, c * DC : (c + 1) * DC].rearrange(
                                        "b d l -> b (d l)"
                                    ),
                                )
                        else:
                            eng.dma_start(
                                dst[:], k_h[trows].rearrange("b d l -> b (d l)")
                            )

                    kbufs = 4 if kb4 else 3
                    if khw or gpoff:
                        # software prefetch: issue tile t+1's keys DMA at the
                        # top of tile t so queue waits never block the load
                        if t == 0:
                            kt = kp.tile([PT, L * D], kdt, tag="keys", bufs=kbufs)
                            _keys_dma(kt, rows, nchunks=16)
                        else:
                            kt = _kt_next[0]
                        if t + 1 < NT:
                            ktn = kp.tile(
                                [PT, L * D], kdt, tag="keys", bufs=kbufs, name="ktn"
                            )
                            _keys_dma(ktn, slice((t + 1) * PT, (t + 2) * PT))
                            _kt_next[0] = ktn
                    else:
                        kt = kp.tile([PT, L * D], kdt, tag="keys", bufs=kbufs)
                        _keys_dma(kt, rows)
                    k3t = kt[:].rearrange("p (d l) -> p d l", d=D)

                    if pipe:
                        if t == 0:
                            _front_next[0] = _front(0, sp, dgp, nc, q_h, wb, eye_t)
                            if pipe2 and NT > 1:
                                _front_next[1] = _front(1, sp, dgp, nc, q_h, wb, eye_t)
                        dga = _front_next[0]
                        if pipe2:
                            _front_next[0] = _front_next[1]
                        dg3 = dga[:].rearrange("p (d j) -> p d j", d=D)
                    else:
                        qt = sp.tile([PT, D], f32, tag="q")
                        nc.sync.dma_start(qt[:], q_h[rows, :])

                        # v = q * W, duplicated into bf16 pairs for dga build
                        vt = sp.tile([PT, D], f32, tag="v")
                        nc.vector.tensor_mul(vt[:], qt[:], wb[:])
                    if pipe:
                        pass
                    elif gpoff:
                        dga = dgp.tile([PT, D * 32], bf16, tag="dg", bufs=3)
                        # gpsimd has no packed-mode alignment constraints:
                        # build the diag blocks straight from f32 v
                        nc.gpsimd.tensor_mul(
                            dga[:].rearrange("p (d j) -> p d j", d=D),
                            eye_t[:].unsqueeze(1).broadcast_to([PT, D, 32]),
                            vt[:].unsqueeze(2).broadcast_to([PT, D, 32]),
                        )
                    else:
                        dga = dgp.tile([PT, D * 32], bf16, tag="dg", bufs=3)
                        v2p = sp.tile([PT, 2 * D], bf16, tag="v2p")
                        v2v = v2p[:].rearrange("p (d two) -> p d two", two=2)
                        if actoff:
                            nc.scalar.activation(v2v[:, :, 0], vt[:], AF.Copy)
                            nc.scalar.activation(v2v[:, :, 1], vt[:], AF.Copy)
                        else:
                            nc.vector.tensor_copy(v2v[:, :, 0], vt[:])
                            nc.vector.tensor_copy(v2v[:, :, 1], vt[:])
                        nc.vector.tensor_mul(
                            dga[:].rearrange("p (d j2 two) -> p d j2 two", d=D, two=2),
                            eye_t[:]
                            .rearrange("p (j2 two) -> p j2 two", two=2)
                            .unsqueeze(1)
                            .broadcast_to([PT, D, 16, 2]),
                            v2v.unsqueeze(2).broadcast_to([PT, D, 16, 2]),
                        )
                    if not pipe:
                        dg3 = dga[:].rearrange("p (d j) -> p d j", d=D)

                    # scores: accumulating 32x32 diag-block matmuls, one PSUM
                    # bank per block so the four streams run concurrently
                    pscs = [
                        pp.tile([PT, L], f32, tag=f"sc{i}", name=f"psc{i}")
                        for i in range(4)
                    ]
                    for d in range(D):
                        for i in range(4):
                            s = slice(32 * i, 32 * i + 32)
                            nc.tensor.matmul(
                                pscs[i][s, :],
                                dg3[s, d, :],
                                k3t[s, d, :],
                                start=(d == 0),
                                stop=(d == D - 1),
                                tile_position=(32 * i, 32 * i),
                            )

                    th = sp.tile([PT, L], f32, tag="th")
                    for i in range(4):
                        s = slice(32 * i, 32 * i + 32)
                        nc.scalar.activation(th[s, :], pscs[i][s, :], AF.Tanh)
                    S = sp.tile([PT, 1], f32, tag="S")
                    pe = sp.tile([PT, L], bf16, tag="pe")
                    nc.scalar.activation(pe[:], th[:], AF.Exp, accum_out=S[:])
                    if pipe and not pipe2 and t + 1 < NT:
                        _front_next[0] = _front(t + 1, sp, dgp, nc, q_h, wb, eye_t)
                    elif pipe2 and t + 2 < NT:
                        _front_next[1] = _front(t + 2, sp, dgp, nc, q_h, wb, eye_t)
                    sinv = sp.tile([PT, 1], f32, tag="sinv")
                    nc.vector.reciprocal(sinv[:], S[:])
                    if pnorm:
                        # normalize the 200 weights once (bf16 4x) instead of
                        # the 64 outputs at the tail
                        pen = sp.tile([PT, L], bf16, tag="pen")
                        nc.vector.tensor_scalar_mul(pen[:], pe[:], sinv[:])
                        pe = pen

                    # wk[p, d, l] = keys * exp(scores); broadcast along outer d
                    wk = wp.tile([PT, L * D], bf16, tag="work")
                    w3t = wk[:].rearrange("p (d l) -> p d l", d=D)
                    if gpoff:
                        DS = D - 8  # gpsimd takes the top 8 d-values
                        nc.vector.tensor_mul(
                            w3t[:, 0:DS, :],
                            k3t[:, 0:DS, :],
                            pe[:].unsqueeze(1).broadcast_to([PT, DS, L]),
                        )
                        nc.gpsimd.tensor_mul(
                            w3t[:, DS:D, :],
                            k3t[:, DS:D, :],
                            pe[:].unsqueeze(1).broadcast_to([PT, D - DS, L]),
                        )
                    else:
                        nc.vector.tensor_mul(
                            w3t,
                            k3t,
                            pe[:].unsqueeze(1).broadcast_to([PT, D, L]),
                        )

                    # out_unnorm[p, d] = sum_l wk, then normalize
                    of = sp.tile([PT, D], f32, tag="of")
                    if rsum:
                        # contiguous bf16 l-runs: one 2x-mode reduce replaces
                        # the whole add-tree (fp32 internal accumulation)
                        ou = sp.tile([PT, D], bf16, tag="ou")
                        with nc.allow_low_precision(
                            reason="bf16 pooled output within 2e-2 tolerance"
                        ):
                            nc.vector.reduce_sum(ou[:], w3t, axis=X)
                    elif rtail:
                        # halve on the tree while runs are big; one reduce_sum
                        # mops up the overhead-dominated small levels
                        ou = sp.tile([PT, D], f32, tag="ou")
                        cur = _tree_reduce_partial(
                            nc, tp, w3t, D, _SCHED_200[:3], bf16, "ltree"
                        )
                        nc.vector.reduce_sum(ou[:], cur, axis=X)
                    elif v2:
                        ou = sp.tile([PT, D], f32, tag="ou")
                        _tree_reduce_inner_sched(
                            nc, tp, w3t, D, _SCHED_200, bf16, ou[:], "ltree"
                        )
                    else:
                        ou = sp.tile([PT, D], f32, tag="ou")
                        _tree_reduce_inner_any(
                            nc, tp, w3t, D, L, bf16, ou[:], "ltree"
                        )
                    if pnorm:
                        nc.sync.dma_start(o_h[rows, :], ou[:])
                    else:
                        nc.vector.tensor_scalar_mul(of[:], ou[:], sinv[:])
                        nc.sync.dma_start(o_h[rows, :], of[:])
                    continue

                kt = kp.tile(
                    [PT, L * D], kdt, tag="keys",
                    bufs=1 if hwcast else (3 if keys_bf16 else 2),
                )
                if keys_bf16 and hwcast:
                    ktf = kp.tile([PT, L * D], f32, tag="keysf")
                    nc.sync.dma_start(
                        ktf[:], k_h[rows].rearrange("b l d -> b (l d)")
                    )
                    nc.vector.tensor_copy(kt[:], ktf[:])
                elif keys_bf16:
                    # SWDGE cast-DMA: f32 HBM -> bf16 SBUF
                    nc.gpsimd.dma_start(
                        kt[:], k_h[rows].rearrange("b l d -> b (l d)")
                    )
                else:
                    nc.sync.dma_start(
                        kt[:], k_h[rows].rearrange("b l d -> b (l d)")
                    )
                qt = sp.tile([PT, D], f32, tag="q")
                nc.sync.dma_start(qt[:], q_h[rows, :])

                k3 = kt[:].rearrange("p (l d) -> p l d", l=L)

                if rk1:
                    # v = q * W kept f32, then duplicated into adjacent bf16
                    # pairs (v2p[2d], v2p[2d+1]) = v[d] for the paired
                    # broadcast below.
                    vt = sp.tile([PT, D], f32, tag="v")
                    nc.vector.tensor_mul(vt[:], qt[:], wb[:])
                    v2p = sp.tile([PT, 2 * D], bf16, tag="v2p")
                    v2v = v2p[:].rearrange("p (d two) -> p d two", two=2)
                    nc.vector.tensor_copy(v2v[:, :, 0], vt[:])
                    nc.vector.tensor_copy(v2v[:, :, 1], vt[:])
                    # Build all 64 diag(v[:, d]) blocks in one 2x-mode TT:
                    # dg_all[p, d, j] = eye[p, j] * v[p, d]
                    ew = 32 if rk4 else PT
                    dga = dgp.tile([PT, D * ew], bf16, tag="dg", bufs=1 if hwcast else 3)
                    nc.vector.tensor_mul(
                        dga[:].rearrange(
                            "p (d j2 two) -> p d j2 two", d=D, two=2
                        ),
                        eye_t[:]
                        .rearrange("p (j2 two) -> p j2 two", two=2)
                        .unsqueeze(1)
                        .broadcast_to([PT, D, ew // 2, 2]),
                        v2v.unsqueeze(2).broadcast_to([PT, D, ew // 2, 2]),
                    )
                    # scores[b, l] = sum_d v[b, d] * keys[b, l, d] as
                    # accumulating rank-1 diag matmuls on the TensorEngine:
                    # lhsT = diag(v[:, d]), rhs = keys[:, :, d]
                    dg3 = dga[:].rearrange("p (d j) -> p d j", d=D)
                    if pbank:
                        # one PSUM bank per 32-row block so the four
                        # tile_position matmuls can stream concurrently
                        pscs = [
                            pp.tile([PT, L], f32, tag=f"sc{i}", name=f"psc{i}")
                            for i in range(4)
                        ]
                        for d in range(D):
                            for i in range(4):
                                s = slice(32 * i, 32 * i + 32)
                                nc.tensor.matmul(
                                    pscs[i][s, :],
                                    dg3[s, d, :],
                                    k3[s, :, d],
                                    start=(d == 0),
                                    stop=(d == D - 1),
                                    tile_position=(32 * i, 32 * i),
                                )
                        scores = pscs
                    else:
                        psc = pp.tile([PT, L], f32, tag="sc")
                        for d in range(D):
                            if rk4:
                                # four concurrent 32x32 diag-block matmuls
                                for i in range(4):
                                    s = slice(32 * i, 32 * i + 32)
                                    nc.tensor.matmul(
                                        psc[s, :],
                                        dg3[s, d, :],
                                        k3[s, :, d],
                                        start=(d == 0),
                                        stop=(d == D - 1),
                                        tile_position=(32 * i, 32 * i),
                                    )
                            else:
                                nc.tensor.matmul(
                                    psc[:],
                                    dg3[:, d, :],
                                    k3[:, :, d],
                                    start=(d == 0),
                                    stop=(d == D - 1),
                                )
                        scores = psc
                else:
                    # v = q * W  (per-partition [128, 64])
                    vt = sp.tile([PT, D], mdt, tag="v")
                    nc.vector.tensor_mul(vt[:], qt[:], wb[:])

                    # inter = keys * v (v broadcast along l)
                    inter = wp.tile([PT, L * D], mdt, tag="work")
                    i3 = inter[:].rearrange("p (l d) -> p l d", l=L)
                    nc.vector.tensor_mul(
                        i3, k3, vt[:].unsqueeze(1).broadcast_to([PT, L, D])
                    )

                    # scores[b, l] = sum_d inter
                    scores = sp.tile([PT, L], f32, tag="sc")
                    if d_tree:
                        _tree_reduce_inner(nc, tp, i3, L, D, mdt, scores[:], "dtree")
                    else:
                        nc.vector.reduce_sum(scores[:], i3, axis=X)

                # tanh then exp (same ACT table set); accumulate softmax denom
                th = sp.tile([PT, L], f32, tag="th")
                if isinstance(scores, list):
                    for i in range(4):
                        s = slice(32 * i, 32 * i + 32)
                        nc.scalar.activation(th[s, :], scores[i][s, :], AF.Tanh)
                else:
                    nc.scalar.activation(th[:], scores[:], AF.Tanh)
                S = sp.tile([PT, 1], f32, tag="S")
                wk = wp.tile([PT, L * D], mdt, tag="work")
                w3 = wk[:].rearrange("p (l d) -> p l d", l=L)
                if pe2:
                    # exp weights duplicated into adjacent pairs so the
                    # broadcast-along-d AP has innermost step 1 (4B-aligned
                    # bf16 pair) -> DVE 2x_1P packed mode for the multiply.
                    ped = sp.tile([PT, 2 * L], mdt, tag="pe")
                    p3 = ped[:].rearrange("p (l two) -> p l two", two=2)
                    nc.scalar.activation(p3[:, :, 0], th[:], AF.Exp, accum_out=S[:])
                    nc.scalar.activation(p3[:, :, 1], th[:], AF.Exp)
                    sinv = sp.tile([PT, 1], f32, tag="sinv")
                    nc.vector.reciprocal(sinv[:], S[:])
                    nc.vector.tensor_mul(
                        wk[:].rearrange("p (l d2 two) -> p l d2 two", l=L, two=2),
                        kt[:].rearrange("p (l d2 two) -> p l d2 two", l=L, two=2),
                        p3.unsqueeze(2).broadcast_to([PT, L, D // 2, 2]),
                    )
                else:
                    pe = sp.tile([PT, L], mdt, tag="pe")
                    nc.scalar.activation(pe[:], th[:], AF.Exp, accum_out=S[:])
                    sinv = sp.tile([PT, 1], f32, tag="sinv")
                    nc.vector.reciprocal(sinv[:], S[:])
                    # wk = keys * exp(scores) (broadcast along d)
                    nc.vector.tensor_mul(
                        w3, k3, pe[:].unsqueeze(2).broadcast_to([PT, L, D])
                    )

                # out_unnorm[b, d] = sum_l wk
                ou = sp.tile([PT, D], f32, tag="ou")
                if l_tree:
                    _tree_reduce_outer(nc, tp, w3, L, D, mdt, ou[:], "ltree")
                else:
                    nc.vector.reduce_sum(
                        ou[:],
                        wk[:].rearrange("p (l d) -> p d l", l=L),
                        axis=X,
                    )
                # normalize by softmax denominator
                of = sp.tile([PT, D], f32, tag="of")
                nc.vector.tensor_scalar_mul(of[:], ou[:], sinv[:])
                nc.sync.dma_start(o_h[rows, :], of[:])

    nc.compile()
    return nc


def _get_nc(variant=DEFAULT_VARIANT):
    key = tuple(variant)
    if key not in _cache:
        _cache[key] = _build_bass(key)
    return _cache[key]


def run_sharded(query, keys, W, trace=False, variant=DEFAULT_VARIANT):
    """Run the SPMD kernel; returns (out [B, D], BassKernelResults)."""
    query = np.ascontiguousarray(query, dtype=np.float32)
    keys = np.ascontiguousarray(keys, dtype=np.float32)
    W = np.ascontiguousarray(W, dtype=np.float32)
    nc = _get_nc(variant)
    if len(variant) > 8 and variant[8]:
        # kT path: keys laid out [B, D, L] in HBM
        keys = np.ascontiguousarray(keys.transpose(0, 2, 1))
        if len(variant) > 12 and variant[12]:
            import ml_dtypes

            keys = keys.astype(ml_dtypes.bfloat16)
    w_b = np.ascontiguousarray(np.broadcast_to(W.reshape(1, D), (PT, D)))
    extra = {}
    if len(variant) > 4 and variant[4]:
        import ml_dtypes

        if len(variant) > 6 and variant[6]:
            e = np.zeros((PT, 32), dtype=ml_dtypes.bfloat16)
            e[np.arange(PT), np.arange(PT) % 32] = 1
            extra["eye"] = e
        else:
            extra["eye"] = np.eye(PT, dtype=ml_dtypes.bfloat16)
    in_maps = [
        {
            "query": query[i * BC : (i + 1) * BC],
            "keys": keys[i * BC : (i + 1) * BC],
            "W": w_b,
            **extra,
        }
        for i in range(NCORES)
    ]
    res = run_bass_kernel_spmd(nc, in_maps, core_ids=list(range(NCORES)), trace=trace)
    out = np.concatenate([res.results[i]["out"] for i in range(NCORES)], axis=0)
    return out, res


def _spot_check(out, query, keys, W, n=512):
    """Scaled absmax error of a row subset vs a float64 numpy oracle."""
    idx = np.random.default_rng(0).choice(B, n, replace=False)
    q = query[idx].astype(np.float64)
    k = keys[idx].astype(np.float64)
    w = W.reshape(-1).astype(np.float64)
    sc = np.tanh(((k * q[:, None, :]) * w).sum(-1))
    p = np.exp(sc)
    p /= p.sum(1, keepdims=True)
    ref = (p[:, :, None] * k).sum(1)
    return np.abs(out[idx] - ref).max() / max(np.abs(ref).max(), 1e-6)


def kernel(query, keys, W):
    var = _run_state["variant"]
    try:
        out, _ = run_sharded(query, keys, W, trace=False, variant=var)
        if var != SAFE_VARIANT and not _run_state["checked"]:
            _run_state["checked"] = True
            if _spot_check(out, query, keys, W) > 2e-2:
                raise RuntimeError("fast-variant accuracy check failed")
    except Exception:
        if var == SAFE_VARIANT:
            raise
        _run_state["variant"] = SAFE_VARIANT
        out, _ = run_sharded(query, keys, W, trace=False, variant=SAFE_VARIANT)
    return out



# revision 35
# speedup vs baseline: 1.0117x; 1.0100x over previous
"""Attention-pooling kernel for Trainium2 (8 NeuronCores, data parallel).

Computes, for full inputs query [B, D], keys [B, L, D], W [1, D]:
    inter  = keys * query[:, None, :]
    scores = tanh(einsum('bld,od->blo', inter, W))
    p      = softmax(scores, axis=1)
    out    = sum(p * keys, axis=1)                      # [B, D]

Sharding: batch dim split evenly across 8 cores; W replicated.

Fast path: keys are transposed to [B, D, L] and cast to bf16 on the host, so
each [128, D*L] SBUF tile has contiguous l-runs. Scores come from 64
accumulating rank-1 diag matmuls (four 32x32 tile_position blocks, one PSUM
bank each, so the block streams run concurrently at full rhs rate); ACT does
tanh/exp (+ softmax denominator via accum_out); DVE does the exp-weight
multiply (outer-dim broadcast, 2x mode) and an alignment-preserving halving
add-tree over l. Keys stream via chunked SWDGE DMAs double-buffered 3 deep;
the next tile's query load and diag build are software-pipelined one tile
ahead so the PE always has weights ready.
A scaled-absmax spot check falls back to the f32 SAFE variant on failure.
"""

import sys

if "/opt/trn_rl_repo" not in sys.path:
    sys.path.insert(0, "/opt/trn_rl_repo")

import numpy as np

import concourse.bacc as bacc
import concourse.bass as bass
import concourse.mybir as mybir
import concourse.tile as tile
from concourse.bass_utils import run_bass_kernel_spmd

B, L, D = 16384, 200, 64
NCORES = 8
BC = B // NCORES  # batch rows per core
PT = 128          # partition tile (batch rows per SBUF tile)
NT = BC // PT     # tiles per core

# variant = (keys_bf16, d_tree, l_tree, pe2, rk1, hwcast, rk4, pbank, kT, v2,
#            dchunk, actoff, kbf, khw, gpoff, rsum, rtail, kb4)
# FAST: d-major host-transposed bf16 keys, 32x32 diag-block PE scoring with
# per-block PSUM banks, chunked SWDGE loads, aligned DVE add-tree.
FAST_VARIANT = (
    True, False, True, False, True, False, True, True, True, True, True,
    False, True, False, False, False, False, False, True,
)
SAFE_VARIANT = (False, False, False)
DEFAULT_VARIANT = SAFE_VARIANT

_cache = {}
_run_state = {"variant": FAST_VARIANT, "checked": False}


def _tree_reduce_outer(nc, pool, src_ap, n_outer, inner, dtype, out_ap, tag):
    """Sum over the OUTER axis of a [PT, n_outer, inner] view via halving
    tensor_tensor adds (inner dim stays contiguous, 2x-mode eligible for
    bf16). Final [PT, inner] f32 result lands in out_ap."""
    cur = src_ap
    n = n_outer
    lvl = 0
    while n > 1:
        h, odd = n // 2, n % 2
        if h + odd == 1:
            nc.vector.tensor_add(
                out_ap.unsqueeze(1), cur[:, 0:1, :], cur[:, 1:2, :]
            )
            return
        # ping-pong tags: level k+1 reads level k, so they must coexist
        t = pool.tile([PT, (h + odd) * inner], dtype, tag=f"{tag}{lvl % 2}")
        dst = t[:].rearrange("p (n i) -> p n i", n=h + odd)
        nc.vector.tensor_add(dst[:, 0:h, :], cur[:, 0:h, :], cur[:, h : 2 * h, :])
        if odd:
            nc.vector.tensor_copy(dst[:, h : h + 1, :], cur[:, 2 * h : n, :])
        cur = dst
        n = h + odd
        lvl += 1


# Halving schedule for n=200 chosen so every level's run stride and second
# operand offset stay 4-byte aligned (bf16), keeping DVE 2x mode: entries are
# (h, n_copy) -> n_next = h + n_copy.
_SCHED_200 = [(100, 0), (50, 0), (24, 2), (12, 2), (6, 2), (4, 0), (2, 0), (1, 0)]


def _tree_reduce_partial(nc, pool, src_ap, outer, sched, dtype, tag):
    """Run the first len(sched) halving levels and return the current
    [PT, outer, n] view for another engine to finish."""
    cur = src_ap
    for lvl, (h, cp) in enumerate(sched):
        t = pool.tile([PT, outer * (h + cp)], dtype, tag=f"{tag}{lvl % 2}")
        dst = t[:].rearrange("p (o i) -> p o i", o=outer)
        nc.vector.tensor_add(dst[:, :, 0:h], cur[:, :, 0:h], cur[:, :, h : 2 * h])
        if cp:
            nc.vector.tensor_copy(
                dst[:, :, h : h + cp], cur[:, :, 2 * h : 2 * h + cp]
            )
        cur = dst
    return cur


def _tree_reduce_inner_sched(nc, pool, src_ap, outer, sched, dtype, out_ap, tag):
    """Like _tree_reduce_inner_any but with an explicit (h, n_copy) level
    schedule keeping all adds 2x-eligible."""
    cur = src_ap
    lvl = 0
    for h, cp in sched:
        if h == 1:
            nc.vector.tensor_add(
                out_ap.unsqueeze(2), cur[:, :, 0:1], cur[:, :, 1:2]
            )
            return
        t = pool.tile([PT, outer * (h + cp)], dtype, tag=f"{tag}{lvl % 2}")
        dst = t[:].rearrange("p (o i) -> p o i", o=outer)
        nc.vector.tensor_add(dst[:, :, 0:h], cur[:, :, 0:h], cur[:, :, h : 2 * h])
        if cp:
            nc.vector.tensor_copy(
                dst[:, :, h : h + cp], cur[:, :, 2 * h : 2 * h + cp]
            )
        cur = dst
        lvl += 1


def _tree_reduce_inner_any(nc, pool, src_ap, outer, n_inner, dtype, out_ap, tag):
    """Sum over the INNER axis of a [PT, outer, n_inner] view via halving
    tensor_tensor adds on contiguous inner slices; odd levels park the
    leftover element with a copy. Final [PT, outer] f32 result in out_ap."""
    cur = src_ap
    n = n_inner
    lvl = 0
    while n > 1:
        h, odd = n // 2, n % 2
        if h + odd == 1:
            nc.vector.tensor_add(
                out_ap.unsqueeze(2), cur[:, :, 0:1], cur[:, :, 1:2]
            )
            return
        t = pool.tile([PT, outer * (h + odd)], dtype, tag=f"{tag}{lvl % 2}")
        dst = t[:].rearrange("p (o i) -> p o i", o=outer)
        nc.vector.tensor_add(dst[:, :, 0:h], cur[:, :, 0:h], cur[:, :, h : 2 * h])
        if odd:
            nc.vector.tensor_copy(dst[:, :, h : h + 1], cur[:, :, 2 * h : n])
        cur = dst
        n = h + odd
        lvl += 1


def _tree_reduce_inner(nc, pool, src_ap, outer, n_inner, dtype, out_ap, tag):
    """Sum over the INNER axis of a [PT, outer, n_inner] view via halving
    tensor_tensor adds on contiguous inner slices. n_inner must be a power
    of two. Final [PT, outer] f32 result lands in out_ap."""
    cur = src_ap
    n = n_inner
    lvl = 0
    while n > 1:
        h = n // 2
        if h == 1:
            nc.vector.tensor_add(
                out_ap.unsqueeze(2), cur[:, :, 0:1], cur[:, :, 1:2]
            )
            return
        t = pool.tile([PT, outer * h], dtype, tag=f"{tag}{lvl % 2}")
        dst = t[:].rearrange("p (o i) -> p o i", o=outer)
        nc.vector.tensor_add(dst, cur[:, :, 0:h], cur[:, :, h:n])
        cur = dst
        n = h
        lvl += 1


def _build_bass(variant):
    keys_bf16, d_tree, l_tree = variant[:3]
    pe2 = variant[3] if len(variant) > 3 else False
    rk1 = variant[4] if len(variant) > 4 else False
    hwcast = variant[5] if len(variant) > 5 else False  # f32 HWDGE load + DVE convert
    rk4 = variant[6] if len(variant) > 6 else False  # 32x32 block-diag tile_position
    pbank = variant[7] if len(variant) > 7 else False  # per-block PSUM banks
    kT = variant[8] if len(variant) > 8 else False  # host-transposed keys [BC, D, L]
    v2 = variant[9] if len(variant) > 9 else False  # aligned tree schedule
    dchunk = variant[10] if len(variant) > 10 else False  # chunked keys DMA
    actoff = variant[11] if len(variant) > 11 else False  # v2p/of on ACT
    kbf = variant[12] if len(variant) > 12 else False  # host-precast bf16 keys in HBM
    khw = variant[13] if len(variant) > 13 else False  # keys via sync HWDGE + prefetch
    gpoff = variant[14] if len(variant) > 14 else False  # gpsimd offloads
    rsum = variant[15] if len(variant) > 15 else False  # single 2x reduce_sum over l
    rtail = variant[16] if len(variant) > 16 else False  # tree to n=26, reduce the rest
    kb4 = variant[17] if len(variant) > 17 else False  # keys pool 4 buffers
    pipe = variant[18] if len(variant) > 18 else False  # hoist next tile's front
    pnorm = variant[19] if len(variant) > 19 else False  # scale exp by 1/S pre-mult
    pipe2 = variant[20] if len(variant) > 20 else False  # front 2 tiles ahead
    dacc = variant[21] if len(variant) > 21 else False  # tree lvl0 on SDMA CCE add
    dacc2 = variant[22] if len(variant) > 22 else False  # CCE add, split tiles
    assert not khw or kbf, "khw needs bf16 keys (no cast)"
    assert not pbank or rk4, "pbank requires rk4 blocks"
    assert not rk1 or keys_bf16, "rank-1 scoring requires bf16 keys"
    assert not kT or (rk1 and rk4 and pbank and not pe2), "kT path fixes the rest"
    assert not (v2 or dchunk or actoff) or kT, "v2 flags build on kT"
    f32 = mybir.dt.float32
    bf16 = mybir.dt.bfloat16
    kdt = bf16 if keys_bf16 else f32
    mdt = bf16 if keys_bf16 else f32  # multiply output dtype
    AF = mybir.ActivationFunctionType
    X = mybir.AxisListType.X

    nc = bacc.Bacc("TRN2", target_bir_lowering=False, debug=False, num_devices=NCORES)
    q_h = nc.declare_dram_parameter("query", [BC, D], f32, isOutput=False)
    k_h = nc.declare_dram_parameter(
        "keys",
        [BC, D, L] if kT else [BC, L, D],
        bf16 if kbf else f32,
        isOutput=False,
    )
    w_h = nc.declare_dram_parameter("W", [PT, D], f32, isOutput=False)
    if rk1:
        e_h = nc.declare_dram_parameter(
            "eye", [PT, 32 if rk4 else PT], bf16, isOutput=False
        )
    o_h = nc.declare_dram_parameter("out", [BC, D], f32, isOutput=True)

    with tile.TileContext(nc) as tc:
        with (
            tc.tile_pool(name="keys", bufs=2) as kp,
            tc.tile_pool(name="work", bufs=2) as wp,
            tc.tile_pool(name="tree", bufs=1) as tp,
            tc.tile_pool(name="small", bufs=2) as sp,
            tc.tile_pool(name="diag", bufs=3) as dgp,
            tc.tile_pool(name="psum", bufs=2, space="PSUM") as pp,
            tc.tile_pool(name="const", bufs=1) as cp,
        ):
            if rk1:
                ew = 32 if rk4 else PT
                eye0 = cp.tile([PT, ew], bf16)
                nc.sync.dma_start(eye0[:], e_h[:])
                eye_t = cp.tile([PT, ew], bf16)
                nc.vector.tensor_copy(eye_t[:], eye0[:])
            # W pre-broadcast to all 128 partitions on the host.
            wb0 = cp.tile([PT, D], f32)
            nc.sync.dma_start(wb0[:], w_h[:])
            # Route through a DVE copy so downstream DVE ops depend on it via
            # program order rather than an extra DMA semaphore wait.
            wb = cp.tile([PT, D], f32)
            nc.vector.tensor_copy(wb[:], wb0[:])

            _kt_next = [None]
            _front_next = [None, None]

            def _front(tt, sp, dgp, nc, q_h, wb, eye_t):
                """q load + diag-block build for tile tt (DVE front ops)."""
                f32 = mybir.dt.float32
                bf16 = mybir.dt.bfloat16
                trows = slice(tt * PT, (tt + 1) * PT)
                qt = sp.tile([PT, D], f32, tag="q", name="qt_f", bufs=3)
                nc.sync.dma_start(qt[:], q_h[trows, :])
                v2p = sp.tile([PT, 2 * D], bf16, tag="v2p", name="v2p_f", bufs=3)
                v2v = v2p[:].rearrange("p (d two) -> p d two", two=2)
                nc.vector.tensor_mul(v2v[:, :, 0], qt[:], wb[:])
                nc.vector.tensor_mul(v2v[:, :, 1], qt[:], wb[:])
                dga = dgp.tile([PT, D * 32], bf16, tag="dg", bufs=3, name="dga_f")
                nc.vector.tensor_mul(
                    dga[:].rearrange("p (d j2 two) -> p d j2 two", d=D, two=2),
                    eye_t[:]
                    .rearrange("p (j2 two) -> p j2 two", two=2)
                    .unsqueeze(1)
                    .broadcast_to([PT, D, 16, 2]),
                    v2v.unsqueeze(2).broadcast_to([PT, D, 16, 2]),
                )
                return dga

            for t in range(NT):
                rows = slice(t * PT, (t + 1) * PT)

                if kT:
                    # --- d-major keys path: keys pre-transposed to [BC, D, L]
                    # on the host, so the matmul rhs and the weighted-sum
                    # multiply both walk contiguous l-runs.
                    def _keys_dma(dst, trows, nchunks=4):
                        eng = nc.sync if khw else nc.gpsimd
                        if dchunk:
                            # chunked DMA: matmuls on early d-chunks start
                            # while later chunks stream in (cuts pipeline fill)
                            DC = D // nchunks
                            for c in range(nchunks):
                                eng.dma_start(
                                    dst[:, c * DC * L : (c + 1) * DC * L],
                                    k_h[trows, c * DC : (c + 1) * DC].rearrange(
                                        "b d l -> b (d l)"
                                    ),
                                )
                        else:
                            eng.dma_start(
                                dst[:], k_h[trows].rearrange("b d l -> b (d l)")
                            )

                    kbufs = 4 if kb4 else 3
                    if khw or gpoff:
                        # software prefetch: issue tile t+1's keys DMA at the
                        # top of tile t so queue waits never block the load
                        if t == 0:
                            kt = kp.tile([PT, L * D], kdt, tag="keys", bufs=kbufs)
                            _keys_dma(kt, rows, nchunks=16)
                        else:
                            kt = _kt_next[0]
                        if t + 1 < NT:
                            ktn = kp.tile(
                                [PT, L * D], kdt, tag="keys", bufs=kbufs, name="ktn"
                            )
                            _keys_dma(ktn, slice((t + 1) * PT, (t + 2) * PT))
                            _kt_next[0] = ktn
                    else:
                        kt = kp.tile([PT, L * D], kdt, tag="keys", bufs=kbufs)
                        _keys_dma(kt, rows)
                    k3t = kt[:].rearrange("p (d l) -> p d l", d=D)

                    if pipe:
                        if t == 0:
                            _front_next[0] = _front(0, sp, dgp, nc, q_h, wb, eye_t)
                            if pipe2 and NT > 1:
                                _front_next[1] = _front(1, sp, dgp, nc, q_h, wb, eye_t)
                        dga = _front_next[0]
                        if pipe2:
                            _front_next[0] = _front_next[1]
                        dg3 = dga[:].rearrange("p (d j) -> p d j", d=D)
                    else:
                        qt = sp.tile([PT, D], f32, tag="q")
                        nc.sync.dma_start(qt[:], q_h[rows, :])

                        # v = q * W, duplicated into bf16 pairs for dga build
                        vt = sp.tile([PT, D], f32, tag="v")
                        nc.vector.tensor_mul(vt[:], qt[:], wb[:])
                    if pipe:
                        pass
                    elif gpoff:
                        dga = dgp.tile([PT, D * 32], bf16, tag="dg", bufs=3)
                        # gpsimd has no packed-mode alignment constraints:
                        # build the diag blocks straight from f32 v
                        nc.gpsimd.tensor_mul(
                            dga[:].rearrange("p (d j) -> p d j", d=D),
                            eye_t[:].unsqueeze(1).broadcast_to([PT, D, 32]),
                            vt[:].unsqueeze(2).broadcast_to([PT, D, 32]),
                        )
                    else:
                        dga = dgp.tile([PT, D * 32], bf16, tag="dg", bufs=3)
                        v2p = sp.tile([PT, 2 * D], bf16, tag="v2p")
                        v2v = v2p[:].rearrange("p (d two) -> p d two", two=2)
                        if actoff:
                            nc.scalar.activation(v2v[:, :, 0], vt[:], AF.Copy)
                            nc.scalar.activation(v2v[:, :, 1], vt[:], AF.Copy)
                        else:
                            nc.vector.tensor_copy(v2v[:, :, 0], vt[:])
                            nc.vector.tensor_copy(v2v[:, :, 1], vt[:])
                        nc.vector.tensor_mul(
                            dga[:].rearrange("p (d j2 two) -> p d j2 two", d=D, two=2),
                            eye_t[:]
                            .rearrange("p (j2 two) -> p j2 two", two=2)
                            .unsqueeze(1)
                            .broadcast_to([PT, D, 16, 2]),
                            v2v.unsqueeze(2).broadcast_to([PT, D, 16, 2]),
                        )
                    if not pipe:
                        dg3 = dga[:].rearrange("p (d j) -> p d j", d=D)

                    # scores: accumulating 32x32 diag-block matmuls, one PSUM
                    # bank per block so the four streams run concurrently
                    pscs = [
                        pp.tile([PT, L], f32, tag=f"sc{i}", name=f"psc{i}")
                        for i in range(4)
                    ]
                    for d in range(D):
                        for i in range(4):
                            s = slice(32 * i, 32 * i + 32)
                            nc.tensor.matmul(
                                pscs[i][s, :],
                                dg3[s, d, :],
                                k3t[s, d, :],
                                start=(d == 0),
                                stop=(d == D - 1),
                                tile_position=(32 * i, 32 * i),
                            )

                    th = sp.tile([PT, L], f32, tag="th")
                    for i in range(4):
                        s = slice(32 * i, 32 * i + 32)
                        nc.scalar.activation(th[s, :], pscs[i][s, :], AF.Tanh)
                    S = sp.tile([PT, 1], f32, tag="S")
                    pe = sp.tile([PT, L], bf16, tag="pe")
                    nc.scalar.activation(pe[:], th[:], AF.Exp, accum_out=S[:])
                    if pipe and not pipe2 and not dacc2 and t + 1 < NT:
                        _front_next[0] = _front(t + 1, sp, dgp, nc, q_h, wb, eye_t)
                    elif pipe2 and t + 2 < NT:
                        _front_next[1] = _front(t + 2, sp, dgp, nc, q_h, wb, eye_t)
                    sinv = sp.tile([PT, 1], f32, tag="sinv")
                    nc.vector.reciprocal(sinv[:], S[:])
                    if pnorm:
                        # normalize the 200 weights once (bf16 4x) instead of
                        # the 64 outputs at the tail
                        pen = sp.tile([PT, L], bf16, tag="pen")
                        nc.vector.tensor_scalar_mul(pen[:], pe[:], sinv[:])
                        pe = pen

                    # wk[p, d, l] = keys * exp(scores); broadcast along outer d
                    if dacc2:
                        # two contiguous half-products; the SDMA inline adder
                        # (CCE) folds them: wka += wkb == tree level 0
                        wka = wp.tile([PT, D * 100], bf16, tag="worka")
                        wkb = wp.tile([PT, D * 100], bf16, tag="workb")
                        w3a = wka[:].rearrange("p (d l) -> p d l", d=D)
                        w3b = wkb[:].rearrange("p (d l) -> p d l", d=D)
                        nc.vector.tensor_mul(
                            w3a,
                            k3t[:, :, 0:100],
                            pe[:, 0:100].unsqueeze(1).broadcast_to([PT, D, 100]),
                        )
                        nc.vector.tensor_mul(
                            w3b,
                            k3t[:, :, 100:200],
                            pe[:, 100:200].unsqueeze(1).broadcast_to([PT, D, 100]),
                        )
                        nc.gpsimd.dma_start(
                            wka[:], wkb[:], accum_op=mybir.AluOpType.add
                        )
                        if pipe and t + 1 < NT:
                            # front ops cover the accumulate-DMA latency
                            _front_next[0] = _front(
                                t + 1, sp, dgp, nc, q_h, wb, eye_t
                            )
                        ou = sp.tile([PT, D], f32, tag="ou")
                        of = sp.tile([PT, D], f32, tag="of")
                        _tree_reduce_inner_sched(
                            nc, tp, w3a, D, _SCHED_200[1:], bf16, ou[:], "ltree"
                        )
                        nc.vector.tensor_scalar_mul(of[:], ou[:], sinv[:])
                        nc.sync.dma_start(o_h[rows, :], of[:])
                        continue
                    wk = wp.tile([PT, L * D], bf16, tag="work")
                    w3t = wk[:].rearrange("p (d l) -> p d l", d=D)
                    if gpoff:
                        DS = D - 8  # gpsimd takes the top 8 d-values
                        nc.vector.tensor_mul(
                            w3t[:, 0:DS, :],
                            k3t[:, 0:DS, :],
                            pe[:].unsqueeze(1).broadcast_to([PT, DS, L]),
                        )
                        nc.gpsimd.tensor_mul(
                            w3t[:, DS:D, :],
                            k3t[:, DS:D, :],
                            pe[:].unsqueeze(1).broadcast_to([PT, D - DS, L]),
                        )
                    else:
                        nc.vector.tensor_mul(
                            w3t,
                            k3t,
                            pe[:].unsqueeze(1).broadcast_to([PT, D, L]),
                        )

                    # out_unnorm[p, d] = sum_l wk, then normalize
                    of = sp.tile([PT, D], f32, tag="of")
                    if dacc:
                        # tree level 0 on the DMA engines' inline adder:
                        # wk[:, :, 0:100] += wk[:, :, 100:200], then the DVE
                        # tree handles the remaining levels
                        nc.gpsimd.dma_start(
                            w3t[:, :, 0:100],
                            w3t[:, :, 100:200],
                            accum_op=mybir.AluOpType.add,
                        )
                        ou = sp.tile([PT, D], f32, tag="ou")
                        _tree_reduce_inner_sched(
                            nc, tp, w3t[:, :, 0:100], D, _SCHED_200[1:], bf16,
                            ou[:], "ltree",
                        )
                    elif rsum:
                        # contiguous bf16 l-runs: one 2x-mode reduce replaces
                        # the whole add-tree (fp32 internal accumulation)
                        ou = sp.tile([PT, D], bf16, tag="ou")
                        with nc.allow_low_precision(
                            reason="bf16 pooled output within 2e-2 tolerance"
                        ):
                            nc.vector.reduce_sum(ou[:], w3t, axis=X)
                    elif rtail:
                        # halve on the tree while runs are big; one reduce_sum
                        # mops up the overhead-dominated small levels
                        ou = sp.tile([PT, D], f32, tag="ou")
                        cur = _tree_reduce_partial(
                            nc, tp, w3t, D, _SCHED_200[:3], bf16, "ltree"
                        )
                        nc.vector.reduce_sum(ou[:], cur, axis=X)
                    elif v2:
                        ou = sp.tile([PT, D], f32, tag="ou")
                        _tree_reduce_inner_sched(
                            nc, tp, w3t, D, _SCHED_200, bf16, ou[:], "ltree"
                        )
                    else:
                        ou = sp.tile([PT, D], f32, tag="ou")
                        _tree_reduce_inner_any(
                            nc, tp, w3t, D, L, bf16, ou[:], "ltree"
                        )
                    if pnorm:
                        nc.sync.dma_start(o_h[rows, :], ou[:])
                    else:
                        nc.vector.tensor_scalar_mul(of[:], ou[:], sinv[:])
                        nc.sync.dma_start(o_h[rows, :], of[:])
                    continue

                kt = kp.tile(
                    [PT, L * D], kdt, tag="keys",
                    bufs=1 if hwcast else (3 if keys_bf16 else 2),
                )
                if keys_bf16 and hwcast:
                    ktf = kp.tile([PT, L * D], f32, tag="keysf")
                    nc.sync.dma_start(
                        ktf[:], k_h[rows].rearrange("b l d -> b (l d)")
                    )
                    nc.vector.tensor_copy(kt[:], ktf[:])
                elif keys_bf16:
                    # SWDGE cast-DMA: f32 HBM -> bf16 SBUF
                    nc.gpsimd.dma_start(
                        kt[:], k_h[rows].rearrange("b l d -> b (l d)")
                    )
                else:
                    nc.sync.dma_start(
                        kt[:], k_h[rows].rearrange("b l d -> b (l d)")
                    )
                qt = sp.tile([PT, D], f32, tag="q")
                nc.sync.dma_start(qt[:], q_h[rows, :])

                k3 = kt[:].rearrange("p (l d) -> p l d", l=L)

                if rk1:
                    # v = q * W kept f32, then duplicated into adjacent bf16
                    # pairs (v2p[2d], v2p[2d+1]) = v[d] for the paired
                    # broadcast below.
                    vt = sp.tile([PT, D], f32, tag="v")
                    nc.vector.tensor_mul(vt[:], qt[:], wb[:])
                    v2p = sp.tile([PT, 2 * D], bf16, tag="v2p")
                    v2v = v2p[:].rearrange("p (d two) -> p d two", two=2)
                    nc.vector.tensor_copy(v2v[:, :, 0], vt[:])
                    nc.vector.tensor_copy(v2v[:, :, 1], vt[:])
                    # Build all 64 diag(v[:, d]) blocks in one 2x-mode TT:
                    # dg_all[p, d, j] = eye[p, j] * v[p, d]
                    ew = 32 if rk4 else PT
                    dga = dgp.tile([PT, D * ew], bf16, tag="dg", bufs=1 if hwcast else 3)
                    nc.vector.tensor_mul(
                        dga[:].rearrange(
                            "p (d j2 two) -> p d j2 two", d=D, two=2
                        ),
                        eye_t[:]
                        .rearrange("p (j2 two) -> p j2 two", two=2)
                        .unsqueeze(1)
                        .broadcast_to([PT, D, ew // 2, 2]),
                        v2v.unsqueeze(2).broadcast_to([PT, D, ew // 2, 2]),
                    )
                    # scores[b, l] = sum_d v[b, d] * keys[b, l, d] as
                    # accumulating rank-1 diag matmuls on the TensorEngine:
                    # lhsT = diag(v[:, d]), rhs = keys[:, :, d]
                    dg3 = dga[:].rearrange("p (d j) -> p d j", d=D)
                    if pbank:
                        # one PSUM bank per 32-row block so the four
                        # tile_position matmuls can stream concurrently
                        pscs = [
                            pp.tile([PT, L], f32, tag=f"sc{i}", name=f"psc{i}")
                            for i in range(4)
                        ]
                        for d in range(D):
                            for i in range(4):
                                s = slice(32 * i, 32 * i + 32)
                                nc.tensor.matmul(
                                    pscs[i][s, :],
                                    dg3[s, d, :],
                                    k3[s, :, d],
                                    start=(d == 0),
                                    stop=(d == D - 1),
                                    tile_position=(32 * i, 32 * i),
                                )
                        scores = pscs
                    else:
                        psc = pp.tile([PT, L], f32, tag="sc")
                        for d in range(D):
                            if rk4:
                                # four concurrent 32x32 diag-block matmuls
                                for i in range(4):
                                    s = slice(32 * i, 32 * i + 32)
                                    nc.tensor.matmul(
                                        psc[s, :],
                                        dg3[s, d, :],
                                        k3[s, :, d],
                                        start=(d == 0),
                                        stop=(d == D - 1),
                                        tile_position=(32 * i, 32 * i),
                                    )
                            else:
                                nc.tensor.matmul(
                                    psc[:],
                                    dg3[:, d, :],
                                    k3[:, :, d],
                                    start=(d == 0),
                                    stop=(d == D - 1),
                                )
                        scores = psc
                else:
                    # v = q * W  (per-partition [128, 64])
                    vt = sp.tile([PT, D], mdt, tag="v")
                    nc.vector.tensor_mul(vt[:], qt[:], wb[:])

                    # inter = keys * v (v broadcast along l)
                    inter = wp.tile([PT, L * D], mdt, tag="work")
                    i3 = inter[:].rearrange("p (l d) -> p l d", l=L)
                    nc.vector.tensor_mul(
                        i3, k3, vt[:].unsqueeze(1).broadcast_to([PT, L, D])
                    )

                    # scores[b, l] = sum_d inter
                    scores = sp.tile([PT, L], f32, tag="sc")
                    if d_tree:
                        _tree_reduce_inner(nc, tp, i3, L, D, mdt, scores[:], "dtree")
                    else:
                        nc.vector.reduce_sum(scores[:], i3, axis=X)

                # tanh then exp (same ACT table set); accumulate softmax denom
                th = sp.tile([PT, L], f32, tag="th")
                if isinstance(scores, list):
                    for i in range(4):
                        s = slice(32 * i, 32 * i + 32)
                        nc.scalar.activation(th[s, :], scores[i][s, :], AF.Tanh)
                else:
                    nc.scalar.activation(th[:], scores[:], AF.Tanh)
                S = sp.tile([PT, 1], f32, tag="S")
                wk = wp.tile([PT, L * D], mdt, tag="work")
                w3 = wk[:].rearrange("p (l d) -> p l d", l=L)
                if pe2:
                    # exp weights duplicated into adjacent pairs so the
                    # broadcast-along-d AP has innermost step 1 (4B-aligned
                    # bf16 pair) -> DVE 2x_1P packed mode for the multiply.
                    ped = sp.tile([PT, 2 * L], mdt, tag="pe")
                    p3 = ped[:].rearrange("p (l two) -> p l two", two=2)
                    nc.scalar.activation(p3[:, :, 0], th[:], AF.Exp, accum_out=S[:])
                    nc.scalar.activation(p3[:, :, 1], th[:], AF.Exp)
                    sinv = sp.tile([PT, 1], f32, tag="sinv")
                    nc.vector.reciprocal(sinv[:], S[:])
                    nc.vector.tensor_mul(
                        wk[:].rearrange("p (l d2 two) -> p l d2 two", l=L, two=2),
                        kt[:].rearrange("p (l d2 two) -> p l d2 two", l=L, two=2),
                        p3.unsqueeze(2).broadcast_to([PT, L, D // 2, 2]),
                    )
                else:
                    pe = sp.tile([PT, L], mdt, tag="pe")
                    nc.scalar.activation(pe[:], th[:], AF.Exp, accum_out=S[:])
                    sinv = sp.tile([PT, 1], f32, tag="sinv")
                    nc.vector.reciprocal(sinv[:], S[:])
                    # wk = keys * exp(scores) (broadcast along d)
                    nc.vector.tensor_mul(
                        w3, k3, pe[:].unsqueeze(2).broadcast_to([PT, L, D])
                    )

                # out_unnorm[b, d] = sum_l wk
                ou = sp.tile([PT, D], f32, tag="ou")
                if l_tree:
                    _tree_reduce_outer(nc, tp, w3, L, D, mdt, ou[:], "ltree")
                else:
                    nc.vector.reduce_sum(
                        ou[:],
                        wk[:].rearrange("p (l d) -> p d l", l=L),
                        axis=X,
                    )
                # normalize by softmax denominator
                of = sp.tile([PT, D], f32, tag="of")
                nc.vector.tensor_scalar_mul(of[:], ou[:], sinv[:])
                nc.sync.dma_start(o_h[rows, :], of[:])

    nc.compile()
    return nc


def _get_nc(variant=DEFAULT_VARIANT):
    key = tuple(variant)
    if key not in _cache:
        _cache[key] = _build_bass(key)
    return _cache[key]


def run_sharded(query, keys, W, trace=False, variant=DEFAULT_VARIANT):
    """Run the SPMD kernel; returns (out [B, D], BassKernelResults)."""
    query = np.ascontiguousarray(query, dtype=np.float32)
    keys = np.ascontiguousarray(keys, dtype=np.float32)
    W = np.ascontiguousarray(W, dtype=np.float32)
    nc = _get_nc(variant)
    if len(variant) > 8 and variant[8]:
        # kT path: keys laid out [B, D, L] in HBM
        keys = np.ascontiguousarray(keys.transpose(0, 2, 1))
        if len(variant) > 12 and variant[12]:
            import ml_dtypes

            keys = keys.astype(ml_dtypes.bfloat16)
    w_b = np.ascontiguousarray(np.broadcast_to(W.reshape(1, D), (PT, D)))
    extra = {}
    if len(variant) > 4 and variant[4]:
        import ml_dtypes

        if len(variant) > 6 and variant[6]:
            e = np.zeros((PT, 32), dtype=ml_dtypes.bfloat16)
            e[np.arange(PT), np.arange(PT) % 32] = 1
            extra["eye"] = e
        else:
            extra["eye"] = np.eye(PT, dtype=ml_dtypes.bfloat16)
    in_maps = [
        {
            "query": query[i * BC : (i + 1) * BC],
            "keys": keys[i * BC : (i + 1) * BC],
            "W": w_b,
            **extra,
        }
        for i in range(NCORES)
    ]
    res = run_bass_kernel_spmd(nc, in_maps, core_ids=list(range(NCORES)), trace=trace)
    out = np.concatenate([res.results[i]["out"] for i in range(NCORES)], axis=0)
    return out, res


def _spot_check(out, query, keys, W, n=512):
    """Scaled absmax error of a row subset vs a float64 numpy oracle."""
    idx = np.random.default_rng(0).choice(B, n, replace=False)
    q = query[idx].astype(np.float64)
    k = keys[idx].astype(np.float64)
    w = W.reshape(-1).astype(np.float64)
    sc = np.tanh(((k * q[:, None, :]) * w).sum(-1))
    p = np.exp(sc)
    p /= p.sum(1, keepdims=True)
    ref = (p[:, :, None] * k).sum(1)
    return np.abs(out[idx] - ref).max() / max(np.abs(ref).max(), 1e-6)


def kernel(query, keys, W):
    var = _run_state["variant"]
    try:
        out, _ = run_sharded(query, keys, W, trace=False, variant=var)
        if var != SAFE_VARIANT and not _run_state["checked"]:
            _run_state["checked"] = True
            if _spot_check(out, query, keys, W) > 2e-2:
                raise RuntimeError("fast-variant accuracy check failed")
    except Exception:
        if var == SAFE_VARIANT:
            raise
        _run_state["variant"] = SAFE_VARIANT
        out, _ = run_sharded(query, keys, W, trace=False, variant=SAFE_VARIANT)
    return out

